# revision 7
# baseline (speedup 1.0000x reference)
"""Cross-attention kernel for 8 Trainium2 NeuronCores (v2).

Problem (hardcoded): B=2, NQ=NKV=2048, QDIM=KVDIM=1024, H=16, HD=64.

Sharding: tensor-parallel over heads - 2 heads per core. Each core computes
its heads' Q/K/V projections, scores, softmax and context for the full
sequence, then an AllToAll reshards context from head-split to token-split
so the output projection is fully local; core j returns output tokens
[j*512, (j+1)*512).

v2 changes vs v1:
- PV matmuls are col-split (tile_position cols 0/64) so both heads' context
  accumulate concurrently in one PSUM bank - 2x PV throughput. The softmax
  denominator (v1's ones-column in V) is instead a pairwise fp16 tree sum
  of the probs tiles on the vector engine (independent in-loop adds, no
  16-deep serial chain), reduced across partitions by one pair of
  ones-stationary matmuls per q-group.
- V projection is computed transposed ([dim, token], weight-stationary like
  K/Q so weight loads pipeline) and flipped to [token, dim] by chained PE
  transposes against an identity stationary. v1 computed V natural directly,
  which re-loads the PE stationary every 128-wide matmul and is LDW-bound.
- Projections stream through the attention loop via a demand-driven op
  queue: each kt pulls proj work in behind the exp-paced inner loop, with
  per-kt deadlines (K tile before scores, V tile before PV, Q before group).
"""

import os
from collections import deque

import numpy as np
import ml_dtypes

import concourse.bass as bass
import concourse.mybir as mybir
import concourse.tile as tile
from concourse import bacc
from concourse.bass_utils import run_bass_kernel_spmd

N_CORES = 8
B = 2
NQ = NKV = 2048
C = 1024          # model dim (QDIM=KVDIM=INNER)
H, HD = 16, 64
T = B * NQ        # 4096 flattened tokens
DL = 128          # local head dims per core (2 heads * 64)
TSH = T // N_CORES  # 512 output tokens per core
SCALE = HD ** -0.5

F32 = mybir.dt.float32
BF16 = mybir.dt.bfloat16
FP16 = mybir.dt.float16

_NC_CACHE = None
_LAST_RESULTS = None


def _build(with_collective=True, reps=None, stop_after=None, probe=None,
           probe_group=(0, 0), serial_proj=False):
    nc = bacc.Bacc("TRN2", target_bir_lowering=False, debug=False,
                   num_devices=N_CORES)

    qT = nc.dram_tensor("qT", [C, T], BF16, kind="ExternalInput")
    kvT = nc.dram_tensor("kvT", [C, T], BF16, kind="ExternalInput")
    wq = nc.dram_tensor("wq", [C, DL], BF16, kind="ExternalInput")
    wk = nc.dram_tensor("wk", [C, DL], BF16, kind="ExternalInput")
    wv = nc.dram_tensor("wv", [C, DL], BF16, kind="ExternalInput")
    wo = nc.dram_tensor("wo", [C, C], BF16, kind="ExternalInput")
    bias = nc.dram_tensor("bias", [C], F32, kind="ExternalInput")
    ident = nc.dram_tensor("ident", [128, 128], BF16, kind="ExternalInput")
    out = nc.dram_tensor("out", [TSH, C], F32, kind="ExternalOutput")
    dbg = {}
    if probe == "proj":
        dbg["K"] = nc.dram_tensor("dbgK", [128, T], F32, kind="ExternalOutput")
        dbg["Q"] = nc.dram_tensor("dbgQ", [128, T], F32, kind="ExternalOutput")
        dbg["VT"] = nc.dram_tensor("dbgVT", [128, T], F32, kind="ExternalOutput")
        dbg["V"] = nc.dram_tensor("dbgV", [128, T // 128, 128], F32,
                                  kind="ExternalOutput")
    elif probe == "end":
        dbg["K"] = nc.dram_tensor("dbgK", [128, T], F32, kind="ExternalOutput")
        dbg["Q"] = nc.dram_tensor("dbgQ", [128, T], F32, kind="ExternalOutput")
        dbg["VT"] = nc.dram_tensor("dbgVT", [128, T], F32, kind="ExternalOutput")
        dbg["V"] = nc.dram_tensor("dbgV", [128, T // 128, 128], F32,
                                  kind="ExternalOutput")
    elif probe == "fin":
        dbg["ctxF"] = nc.dram_tensor("dbgctxF", [128, N_CORES, TSH], F32,
                                     kind="ExternalOutput")
        dbg["bias"] = nc.dram_tensor("dbgbias", [128, C], F32,
                                     kind="ExternalOutput")
    elif probe == "attn0":
        dbg["prall"] = nc.dram_tensor("dbgprall", [16, 128, 1024], F32,
                                      kind="ExternalOutput")
        dbg["Kd"] = nc.dram_tensor("dbgKd", [128, T], F32,
                                   kind="ExternalOutput")
        dbg["pr"] = nc.dram_tensor("dbgpr", [128, 1024], F32,
                                   kind="ExternalOutput")
        dbg["dacc"] = nc.dram_tensor("dbgdacc", [128, 1024], F32,
                                     kind="ExternalOutput")
        dbg["D"] = nc.dram_tensor("dbgD", [33, 512], F32,
                                  kind="ExternalOutput")
        dbg["ctx"] = nc.dram_tensor("dbgctx", [128, 512], F32,
                                    kind="ExternalOutput")
        dbg["bc"] = nc.dram_tensor("dbgbc", [128, 512], F32,
                                   kind="ExternalOutput")
        dbg["ctxn"] = nc.dram_tensor("dbgctxn", [128, 512], F32,
                                     kind="ExternalOutput")

    CC = C // 128    # 8 contraction chunks
    KT = NKV // 128  # 16 k-tiles per batch
    NT = T // 512    # 8 projection tiles of 512 tokens
    Exp = mybir.ActivationFunctionType.Exp

    with tile.TileContext(nc) as tc:
        with (
            tc.tile_pool(name="consts", bufs=1) as consts,
            tc.tile_pool(name="xt", bufs=3) as xt,
            tc.tile_pool(name="probs", bufs=6) as probs_p,
            tc.tile_pool(name="accp", bufs=2) as accp,
            tc.tile_pool(name="norm", bufs=2) as norm,
            tc.tile_pool(name="outp", bufs=2) as outp,
            tc.tile_pool(name="dram", bufs=1, space="DRAM") as dram,
        ):
            # ---- constants ----
            wq_sb = consts.tile([128, CC, DL], BF16)
            nc.sync.dma_start(out=wq_sb, in_=wq.ap().rearrange("(n p) d -> p n d", p=128))
            wk_sb = consts.tile([128, CC, DL], BF16)
            nc.sync.dma_start(out=wk_sb, in_=wk.ap().rearrange("(n p) d -> p n d", p=128))
            wv_sb = consts.tile([128, CC, DL], BF16)
            nc.sync.dma_start(out=wv_sb, in_=wv.ap().rearrange("(n p) d -> p n d", p=128))
            wo_sb = consts.tile([128, CC, C], BF16)
            nc.sync.dma_start(out=wo_sb, in_=wo.ap().rearrange("(n p) e -> p n e", p=128))
            bias_sb = consts.tile([128, C], F32)
            bias_bc = bass.AP(tensor=bias, offset=0, ap=[[0, 128], [1, C]])
            nc.gpsimd.dma_start(out=bias_sb[:], in_=bias_bc)
            ones_sb = consts.tile([128, 1], FP16)
            nc.vector.memset(ones_sb, 1.0)
            ident_sb = consts.tile([128, 128], BF16)
            nc.sync.dma_start(out=ident_sb, in_=ident.ap())

            # persistent activations
            Kd_sb = consts.tile([128, T], BF16)    # K^T: [d_local, token]
            Qd_sb = consts.tile([128, T], BF16)    # Q^T: [d_local, token]
            VT_sb = consts.tile([128, T], BF16)    # V^T: [d_local, token]
            # V natural, [token-part, 32 tiles, 128 dims] (h0 0:64, h1 64:128)
            V_sb = consts.tile([128, T // 128, 128], BF16)

            qT_r = qT.ap().rearrange("(n p) t -> p n t", p=128)
            kvT_r = kvT.ap().rearrange("(n p) t -> p n t", p=128)

            def _body(_it=None):
                with tc.tile_pool(name="ps", bufs=1, space="PSUM") as ps:
                    # ---------- projection op stream ----------
                    # Each op is a closure; labels mark completion points.
                    # Layout of one tile tt (512 tokens):
                    #   dma(kv), K mms+copy, V mms+copy+xbar-transpose,
                    #   [dma(q), Q mms+copy]
                    ops = deque()        # (label_or_None, fn)
                    done = set()

                    def emit_until(label):
                        while label not in done:
                            lab, fn = ops.popleft()
                            fn()
                            if lab:
                                done.add(lab)

                    def pump(n):
                        for _ in range(n):
                            if not ops:
                                return
                            lab, fn = ops.popleft()
                            fn()
                            if lab:
                                done.add(lab)

                    def drain():
                        while ops:
                            lab, fn = ops.popleft()
                            fn()
                            if lab:
                                done.add(lab)

                    state = {}

                    def dma_kv(tt):
                        def f():
                            t0 = tt * 512
                            # bufs=5: a slot's previous readers retire a
                            # full batch before the ring wraps. At bufs=3
                            # the wrap's write-after-read raced on HW and
                            # corrupted in-flight projection reads.
                            kvt = xt.tile([128, CC, 512], BF16, tag="kvt",
                                          name="kvt", bufs=5)
                            nc.sync.dma_start(out=kvt,
                                              in_=kvT_r[:, :, t0:t0 + 512])
                            state[("kvt", tt)] = kvt
                        return (None, f)

                    def dma_q(tt):
                        def f():
                            t0 = tt * 512
                            qt = xt.tile([128, CC, 512], BF16, tag="qt",
                                         name="qt")
                            nc.sync.dma_start(out=qt,
                                              in_=qT_r[:, :, t0:t0 + 512])
                            state[("qt", tt)] = qt
                        return (None, f)

                    def proj_mms(tt, which, w_sb, cclo, cchi, last=False):
                        # one accumulation chunk of the [128,512] psum
                        def f():
                            src = state[("qt" if which == "q" else "kvt", tt)]
                            key = ("pp", which, tt)
                            if cclo == 0:
                                state[key] = ps.tile([128, 512], F32,
                                                     tag="pp", name="pp",
                                                     bufs=2)
                            pp = state[key]
                            for cc in range(cclo, cchi):
                                nc.tensor.matmul(pp, lhsT=w_sb[:, cc, :],
                                                 rhs=src[:, cc, :],
                                                 start=(cc == 0),
                                                 stop=(cc == CC - 1))
                        return (None, f)

                    def proj_copy(tt, which, dst_sb):
                        lab = ("VT" if which == "v" else which.upper(), tt)

                        def f():
                            t0 = tt * 512
                            pp = state.pop(("pp", which, tt))
                            nc.vector.tensor_copy(out=dst_sb[:, t0:t0 + 512],
                                                  in_=pp)
                        return (lab, f)

                    def v_flip(tt):
                        # flip V^T -> V natural on the PE (identity
                        # stationary, loaded once; transposes chain at
                        # ~1 col/cycle). An XBAR dma transpose would be
                        # cheaper but SBUF->SBUF DMAs complete out of
                        # order vs DRAM loads on the shared HWDGE
                        # semaphores, breaking cumulative waits.
                        lab = ("V", tt)

                        def f():
                            t0 = tt * 512
                            pt = ps.tile([128, 512], BF16, tag="pp",
                                         name="vflip", bufs=2)
                            for j in range(4):
                                nc.tensor.transpose(
                                    pt[:, j * 128:(j + 1) * 128],
                                    VT_sb[:, t0 + j * 128:t0 + (j + 1) * 128],
                                    ident_sb)
                            nc.vector.tensor_copy(
                                out=V_sb[:, tt * 4:(tt + 1) * 4, :], in_=pt)
                        return (lab, f)

                    def push_kv(tt):
                        ops.append(dma_kv(tt))
                        for c in range(0, CC, 2):
                            ops.append(proj_mms(tt, "k", wk_sb, c, c + 2))
                        ops.append(proj_copy(tt, "k", Kd_sb))
                        for c in range(0, CC, 2):
                            ops.append(proj_mms(tt, "v", wv_sb, c, c + 2))
                        ops.append(proj_copy(tt, "v", VT_sb))
                        ops.append(v_flip(tt))

                    def push_q(tt):
                        ops.append(dma_q(tt))
                        for c in range(0, CC, 2):
                            ops.append(proj_mms(tt, "q", wq_sb, c, c + 2))
                        ops.append(proj_copy(tt, "q", Qd_sb))

                    # stream order: batch0 KV, Q0; rest of batch0 Q;
                    # batch1 KV; batch1 Q
                    push_kv(0)
                    push_q(0)
                    for tt in range(1, 4):
                        push_kv(tt)
                    for tt in range(1, 4):
                        push_q(tt)
                    for tt in range(4, 8):
                        push_kv(tt)
                    for tt in range(4, 8):
                        push_q(tt)

                    if probe == "proj":
                        drain()
                        nc.gpsimd.dma_start(out=dbg["K"].ap(), in_=Kd_sb[:])
                        nc.gpsimd.dma_start(out=dbg["Q"].ap(), in_=Qd_sb[:])
                        nc.gpsimd.dma_start(out=dbg["VT"].ap(), in_=VT_sb[:])
                        nc.gpsimd.dma_start(out=dbg["V"].ap(), in_=V_sb[:])
                        ob0 = outp.tile([128, C], F32, tag="ob", name="ob")
                        nc.vector.memset(ob0, 0.0)
                        for m in range(TSH // 128):
                            nc.sync.dma_start(
                                out=out.ap()[m * 128:(m + 1) * 128, :], in_=ob0)
                        return

                    if serial_proj or stop_after == "proj":
                        drain()
                        if stop_after == "proj":
                            return

                    # ---------- attention ----------
                    a2a_in = dram.tile([N_CORES, DL, TSH], BF16)
                    a2a_out = dram.tile([N_CORES, DL, TSH], BF16)

                    def attn_group(b, qv, budget=1):
                        q0 = b * NQ + qv * 512
                        j = b * 4 + qv
                        emit_until(("Q", b * 4 + qv))
                        emit_until(("K", b * 4))

                        def scores(kt):
                            k0 = b * NKV + kt * 128
                            pair = ps.tile([128, 1024], F32, tag="sc",
                                           name="sc", bufs=2)
                            for h in range(2):
                                hs = slice(h * 64, (h + 1) * 64)
                                nc.tensor.matmul(
                                    pair[:, h * 512:(h + 1) * 512],
                                    lhsT=Kd_sb[hs, k0:k0 + 128],
                                    rhs=Qd_sb[hs, q0:q0 + 512],
                                    start=True, stop=True)
                            return pair

                        psc = ps.tile([128, 512], F32, tag="cx", name="cx",
                                      bufs=2)

                        def pv(kt, pr):
                            vt = b * KT + kt
                            for h in range(2):
                                # interleaved accumulation groups share the
                                # bank (disjoint partitions; has_written is
                                # per element on silicon)
                                nc.tensor.matmul(
                                    psc[h * 64:(h + 1) * 64, :],
                                    lhsT=V_sb[:, vt, h * 64:(h + 1) * 64],
                                    rhs=pr[:, h * 512:(h + 1) * 512],
                                    start=(kt == 0), stop=(kt == KT - 1),
                                    skip_group_check=True)

                        pair = scores(0)
                        prv = None
                        pairsums = []
                        quads = []
                        for kt in range(KT):
                            pr = probs_p.tile([128, 1024], BF16, tag="pr",
                                              name="pr", bufs=12)
                            nc.scalar.activation(out=pr, in_=pair, func=Exp,
                                                 scale=SCALE)
                            if probe == "attn0" and (b, qv) == probe_group:
                                if kt == 0:
                                    nc.gpsimd.dma_start(out=dbg["pr"].ap(),
                                                        in_=pr[:])
                                nc.gpsimd.dma_start(out=dbg["prall"].ap()[kt],
                                                    in_=pr[:])
                            if kt + 1 < KT:
                                emit_until(("K", b * 4 + (kt + 1) // 4))
                                pair = scores(kt + 1)
                            emit_until(("V", b * 4 + kt // 4))
                            pv(kt, pr)
                            if kt % 2 == 0:
                                prv = pr
                            else:
                                pa = accp.tile([128, 1024], FP16, tag="pa",
                                               name="pa", bufs=3)
                                nc.vector.tensor_add(pa, prv, pr)
                                pairsums.append(pa)
                                if kt % 4 == 3:
                                    qa = accp.tile([128, 1024], FP16,
                                                   tag="qa", name="qa",
                                                   bufs=4)
                                    nc.vector.tensor_add(qa, pairsums[-2],
                                                         pairsums[-1])
                                    quads.append(qa)
                            pump(budget)

                        # denominator: 3 more tree levels (depth 4 total,
                        # no 16-deep serial chain), then partition-sum via
                        # ones matmuls sharing the pp psum ring
                        o1 = accp.tile([128, 1024], FP16, tag="oa",
                                       name="oa", bufs=2)
                        nc.vector.tensor_add(o1, quads[0], quads[1])
                        o2 = accp.tile([128, 1024], FP16, tag="oa",
                                       name="oa", bufs=2)
                        nc.vector.tensor_add(o2, quads[2], quads[3])
                        dacc = accp.tile([128, 1024], FP16, tag="da",
                                         name="da", bufs=1)
                        nc.vector.tensor_add(dacc, o1, o2)
                        D = ps.tile([33, 512], F32, tag="pp", name="pd",
                                    bufs=2)
                        nc.tensor.matmul(D[0:1, :], lhsT=ones_sb,
                                         rhs=dacc[:, 0:512],
                                         start=True, stop=True,
                                         skip_group_check=True)
                        nc.tensor.matmul(D[32:33, :], lhsT=ones_sb,
                                         rhs=dacc[:, 512:1024],
                                         start=True, stop=True,
                                         tile_position=(0, 32),
                                         skip_group_check=True)
                        # both heads' reciprocals side by side on partition 0
                        # (partition_broadcast only replicates from base 0),
                        # then per-head muls pick the matching half
                        rr = norm.tile([1, 1024], F32, tag="rr", name="rr",
                                       bufs=1)
                        nc.vector.reciprocal(out=rr[:, 0:512], in_=D[0:1, :])
                        nc.vector.reciprocal(out=rr[:, 512:1024],
                                             in_=D[32:33, :])
                        bc = norm.tile([128, 1024], F32, tag="bc", name="bc",
                                       bufs=1)
                        nc.gpsimd.partition_broadcast(bc[:], rr[:])
                        ctxn = norm.tile([128, 512], BF16, tag="ctxn",
                                         name="ctxn")
                        nc.vector.tensor_mul(ctxn[0:64, :], psc[0:64, :],
                                             bc[0:64, 0:512])
                        nc.vector.tensor_mul(ctxn[64:128, :], psc[64:128, :],
                                             bc[64:128, 512:1024])
                        nc.sync.dma_start(out=a2a_in[j], in_=ctxn)
                        if probe == "attn0" and (b, qv) == probe_group:
                            nc.gpsimd.dma_start(out=dbg["dacc"].ap(),
                                                in_=dacc[:])
                            Dsb = norm.tile([33, 512], F32, tag="Dsb",
                                            name="Dsb")
                            nc.vector.tensor_copy(out=Dsb, in_=D)
                            nc.sync.dma_start(out=dbg["D"].ap(), in_=Dsb)
                            ctxsb = norm.tile([128, 512], F32, tag="ctxsb",
                                              name="ctxsb")
                            nc.vector.tensor_copy(out=ctxsb, in_=psc)
                            nc.sync.dma_start(out=dbg["ctx"].ap(), in_=ctxsb)
                            nc.gpsimd.dma_start(out=dbg["Kd"].ap(),
                                                in_=Kd_sb[:])
                            nc.gpsimd.dma_start(out=dbg["bc"].ap(), in_=bc[:])
                            nc.gpsimd.dma_start(out=dbg["ctxn"].ap(),
                                                in_=ctxn[:])

                    for b in range(B):
                        for qv in range(4):
                            attn_group(b, qv, budget=1)
                            if probe == "attn0" and (b, qv) == probe_group:
                                return
                    drain()
                    if probe == "end":
                        nc.gpsimd.dma_start(out=dbg["K"].ap(), in_=Kd_sb[:])
                        nc.gpsimd.dma_start(out=dbg["Q"].ap(), in_=Qd_sb[:])
                        nc.gpsimd.dma_start(out=dbg["VT"].ap(), in_=VT_sb[:])
                        nc.gpsimd.dma_start(out=dbg["V"].ap(), in_=V_sb[:])
                        return
                    if stop_after == "attn":
                        return

                    if with_collective:
                        nc.gpsimd.collective_compute(
                            "AllToAll", mybir.AluOpType.bypass,
                            replica_groups=[list(range(N_CORES))],
                            ins=[a2a_in.opt()], outs=[a2a_out.opt()])

                    # ---- output projection (local tokens only) ----
                    src = a2a_out if with_collective else a2a_in
                    ctxF = outp.tile([128, N_CORES, TSH], BF16)
                    for i in range(N_CORES):
                        nc.sync.dma_start(out=ctxF[:, i, :], in_=src[i])
                    if probe == "fin":
                        nc.gpsimd.dma_start(out=dbg["ctxF"].ap(), in_=ctxF[:])
                        nc.gpsimd.dma_start(out=dbg["bias"].ap(),
                                            in_=bias_sb[:])
                    for m in range(TSH // 128):
                        ob = outp.tile([128, C], F32, tag="ob", name="ob")
                        for half in range(2):
                            pso = ps.tile([128, 512], F32, tag="pp",
                                          name="pso", bufs=2)
                            for i in range(N_CORES):
                                nc.tensor.matmul(
                                    pso, lhsT=ctxF[:, i, m * 128:(m + 1) * 128],
                                    rhs=wo_sb[:, i, half * 512:(half + 1) * 512],
                                    start=(i == 0), stop=(i == N_CORES - 1))
                            nc.vector.tensor_add(ob[:, half * 512:(half + 1) * 512],
                                                 pso,
                                                 bias_sb[:, half * 512:(half + 1) * 512])
                        nc.sync.dma_start(out=out.ap()[m * 128:(m + 1) * 128, :],
                                          in_=ob)

            if reps is None:
                _body()
            else:
                with tc.For_i(0, reps, 1) as _it:
                    _body(_it)
    nc.compile()
    return nc


def _get_nc():
    global _NC_CACHE
    if _NC_CACHE is None:
        _NC_CACHE = _build()
    return _NC_CACHE


def prep_in_maps(query, key_value, w_q, w_kv, w_out, b_out):
    bf = ml_dtypes.bfloat16
    q2 = np.asarray(query, np.float32).reshape(T, C)
    kv2 = np.asarray(key_value, np.float32).reshape(T, C)
    qT_a = np.ascontiguousarray(q2.T).astype(bf)
    kvT_a = np.ascontiguousarray(kv2.T).astype(bf)
    wo_a = np.asarray(w_out, np.float32).astype(bf)
    bias_a = np.asarray(b_out, np.float32)
    ident_a = np.eye(128, dtype=ml_dtypes.bfloat16)

    in_maps = []
    for j in range(N_CORES):
        cs = slice(j * DL, (j + 1) * DL)
        in_maps.append({
            "qT": qT_a,
            "kvT": kvT_a,
            "wq": np.ascontiguousarray(np.asarray(w_q, np.float32)[:, cs]).astype(bf),
            "wk": np.ascontiguousarray(np.asarray(w_kv, np.float32)[:, cs]).astype(bf),
            "wv": np.ascontiguousarray(
                np.asarray(w_kv, np.float32)[:, C + j * DL: C + (j + 1) * DL]).astype(bf),
            "wo": wo_a,
            "bias": bias_a,
            "ident": ident_a,
        })
    return in_maps


def kernel(query, key_value, w_q, w_kv, w_out, b_out):
    global _LAST_RESULTS
    in_maps = prep_in_maps(query, key_value, w_q, w_kv, w_out, b_out)
    nc = _get_nc()
    res = run_bass_kernel_spmd(nc, in_maps, core_ids=list(range(N_CORES)))
    _LAST_RESULTS = res
    full = np.concatenate([res.results[j]["out"] for j in range(N_CORES)], axis=0)
    return full.reshape(B, NQ, C)
